# revision 2
# baseline (speedup 1.0000x reference)
"""GatNet on Trainium2, 8 NeuronCores — fused on-device GAT.

Device (one SPMD Bass program, Tile framework):
  phase A: H1 = x @ [W1|Wl1|Wr1] sharded by node rows -> AllGather -> T1 (bf16)
  phase B: per dst-node tile (padded CSR, degree-sorted): indirect-gather
           T1[src] rows, edge softmax (no max-sub; scores are O(1)),
           weighted sum on DVE, bias+relu -> g tiles; PE-transpose -> gT
  phase C: H2 = g @ [W2|Wl2|Wr2] from gT -> AllGather -> T2 (bf16)
  phase D: same as B with one head -> g2 tiles -> indirect scatter into a
           graph-grouped padded layout g2p
  phase E: per 128-row block of g2p: PE transpose + free-dim max ->
           per-block pooling partials (output)

Host: CSR/degree schedule, text CNN, fusion MLP head.
"""
import numpy as np

P = 128
BIG = -60000.0


class Cfg:
    def __init__(self, n, e, b, ncores=8):
        self.N = n
        self.E = e
        self.B = b
        self.NC = ncores
        assert n % ncores == 0
        self.NLOC = n // ncores            # dst nodes per core
        self.RPC = ((self.NLOC + P - 1) // P) * P   # padded rows per core
        self.NT = self.RPC // P            # node tiles per core
        self.NPAD = self.RPC * ncores      # T1 rows
        self.PAD1 = self.NPAD - 1          # layer-1 pad slot (zero row, el:=BIG)
        self.PAD2 = self.NPAD              # layer-2 pad slot (extra row in T2)
        self.NB = self.NT + b              # pooling blocks per core
        self.W1C = 390                     # [h(384)|el(3)|er(3)]
        self.W2C = 386                     # [h2(384)|el2|er2]
        self.D = 128
        self.H = 3
        self.HD = 384


def _schedule(cfg, src, dst, graph_ids):
    """Host preprocessing -> per-core device index inputs + host-side maps."""
    N, NC, NLOC, RPC, NT = cfg.N, cfg.NC, cfg.NLOC, cfg.RPC, cfg.NT
    deg = np.bincount(dst, minlength=N)
    order = np.argsort(dst, kind="stable")
    src_s = np.asarray(src, np.int64)[order]
    starts = np.zeros(N + 1, np.int64)
    starts[1:] = np.cumsum(deg)

    permlist = np.full((NC, RPC), -1, np.int64)
    for c in range(NC):
        v0 = c * NLOC
        dl = deg[v0:v0 + NLOC]
        perm = np.argsort(-dl, kind="stable")
        permlist[c, :NLOC] = v0 + perm

    degp = np.where(permlist >= 0, deg[np.maximum(permlist, 0)], 0)
    Kt = degp.reshape(NC, NT, P).max(axis=2).max(axis=0)
    Kt = np.maximum(Kt, 1).astype(np.int64)
    assert int(Kt.max()) * cfg.W1C * 2 <= 65535, f"Kt max {Kt.max()} too large"
    offs = np.zeros(NT + 1, np.int64)
    offs[1:] = np.cumsum(Kt)
    S = int(offs[-1])

    permpos = np.zeros(N, np.int64)
    for c in range(NC):
        pl = permlist[c, :NLOC]
        permpos[pl] = np.arange(NLOC)
    t2row = (np.arange(N) // NLOC) * RPC + permpos

    NB = cfg.NB
    junk = NB * P
    prow = np.zeros((NC, NLOC), np.int64)
    block_graph = np.full((NC, NB), -1, np.int64)
    gia = np.asarray(graph_ids, np.int64)
    for c in range(NC):
        gid = gia[c * NLOC:(c + 1) * NLOC]
        chg = np.flatnonzero(np.diff(gid)) + 1
        seg_starts = np.concatenate([[0], chg])
        seg_ends = np.concatenate([chg, [NLOC]])
        base = 0
        for s0, s1 in zip(seg_starts, seg_ends):
            g = int(gid[s0])
            nblk = (s1 - s0 + P - 1) // P
            prow[c, s0:s1] = base + np.arange(s1 - s0)
            block_graph[c, base // P: base // P + nblk] = g
            base += nblk * P
        assert base <= NB * P, f"pooling overflow core {c}"

    slots1 = np.full((NC, P, S), cfg.PAD1, np.int32)
    slots2 = np.full((NC, P, S), cfg.PAD2, np.int32)
    er1idx = np.full((NC, P, NT), cfg.PAD1, np.int32)
    g2sidx = np.full((NC, P, NT), junk, np.int32)
    jcols = np.arange(int(Kt.max()))[None, :]
    for c in range(NC):
        for t in range(NT):
            nodes = permlist[c, t * P:(t + 1) * P]
            valid = nodes >= 0
            nn = np.maximum(nodes, 0)
            lens = np.where(valid, deg[nn], 0)
            w = int(Kt[t])
            jj = jcols[:, :w]
            m = jj < lens[:, None]
            idx_flat = starts[nn][:, None] + jj
            sub = np.full((P, w), cfg.PAD1, np.int64)
            sub[m] = src_s[idx_flat[m]]
            slots1[c, :, offs[t]:offs[t + 1]] = sub
            sub2 = np.full((P, w), cfg.PAD2, np.int64)
            sub2[m] = t2row[sub[m]]
            slots2[c, :, offs[t]:offs[t + 1]] = sub2
            er1idx[c, valid, t] = nodes[valid]
            loc = nn - c * NLOC
            g2sidx[c, valid, t] = prow[c, loc[valid]]
    # chunk steps: (tile, col_offset, chunk_len, is_first, is_last)
    CK = 32
    steps = []
    for t in range(NT):
        K = int(Kt[t])
        o = int(offs[t])
        pos = 0
        while pos < K:
            ck = min(CK, K - pos)
            steps.append((t, o + pos, ck, pos == 0, pos + ck >= K))
            pos += ck
    return {
        "slots1": slots1, "slots2": slots2, "er1idx": er1idx,
        "g2sidx": g2sidx, "Kt": Kt, "offs": offs, "S": S, "CK": CK,
        "steps": tuple(steps),
        "block_graph": block_graph, "permlist": permlist,
    }


def _build_program(cfg, sched, debug=False):
    import contextlib
    import concourse.bass as bass
    import concourse.tile as tile
    from concourse import bacc, mybir
    from concourse.masks import make_identity

    f32, bf16, i32 = mybir.dt.float32, mybir.dt.bfloat16, mybir.dt.int32
    ADD, MUL, MAX = mybir.AluOpType.add, mybir.AluOpType.mult, mybir.AluOpType.max
    AX = mybir.AxisListType.X
    EXP, RELU = mybir.ActivationFunctionType.Exp, mybir.ActivationFunctionType.Relu
    NT, RPC, NPAD, NB = cfg.NT, cfg.RPC, cfg.NPAD, cfg.NB
    W1C, W2C, HD, H = cfg.W1C, cfg.W2C, cfg.HD, cfg.H
    S, CK, steps = sched["S"], sched["CK"], sched["steps"]

    nc = bacc.Bacc("TRN2", target_bir_lowering=False, debug=False,
                   num_devices=cfg.NC)
    xT = nc.dram_tensor("xT", [P, RPC], bf16, kind="ExternalInput")
    wc1 = nc.dram_tensor("wc1", [P, W1C], bf16, kind="ExternalInput")
    wc2 = nc.dram_tensor("wc2", [HD, W2C], bf16, kind="ExternalInput")
    b1r = nc.dram_tensor("b1r", [P, HD], f32, kind="ExternalInput")
    b2r = nc.dram_tensor("b2r", [P, HD], f32, kind="ExternalInput")
    slots1 = nc.dram_tensor("slots1", [P, S], i32, kind="ExternalInput")
    slots2 = nc.dram_tensor("slots2", [P, S], i32, kind="ExternalInput")
    er1i = nc.dram_tensor("er1i", [P, NT], i32, kind="ExternalInput")
    g2si = nc.dram_tensor("g2si", [P, NT], i32, kind="ExternalInput")
    gp_out = nc.dram_tensor("gp", [P, 3 * NB], f32, kind="ExternalOutput")
    if debug:
        dbgT1 = nc.dram_tensor("dbgT1", [NPAD, W1C], bf16, kind="ExternalOutput")
        dbgT2 = nc.dram_tensor("dbgT2", [NPAD + 1, W2C], bf16, kind="ExternalOutput")
        dbgG = nc.dram_tensor("dbgG", [HD, RPC], bf16, kind="ExternalOutput")
        dbgP = nc.dram_tensor("dbgP", [NB * P + P, HD], bf16, kind="ExternalOutput")

    groups = [list(range(cfg.NC))]

    def ap3(t, off, dims):
        base = t[:, :]
        return bass.AP(base.tensor, off,
                       [tuple(base.ap[0])] + [tuple(d) for d in dims])

    def dap(t, off, dims):
        base = t[:, :]
        return bass.AP(base.tensor, off, [tuple(d) for d in dims])

    with tile.TileContext(nc) as tc:
        with contextlib.ExitStack() as ctx:
            dram = ctx.enter_context(tc.tile_pool(name="dram", bufs=1, space="DRAM"))
            t1loc = dram.tile([RPC, W1C], bf16)
            T1 = dram.tile([NPAD, W1C], bf16)
            gT_d = dram.tile([HD, RPC], bf16)
            t2loc = dram.tile([RPC, W2C], bf16)
            T2 = dram.tile([NPAD + 1, W2C], bf16)
            g2p = dram.tile([NB * P + P, HD], bf16)

            # ---- static SBUF pools (live for the whole kernel) ----
            const = ctx.enter_context(tc.tile_pool(name="const", bufs=1))
            io = ctx.enter_context(tc.tile_pool(name="io", bufs=1))
            pg = ctx.enter_context(tc.tile_pool(name="pg", bufs=2))       # gathered chunks
            pw = ctx.enter_context(tc.tile_pool(name="pw", bufs=1))       # products
            psc = ctx.enter_context(tc.tile_pool(name="psc", bufs=2))     # per-chunk scratch
            pacc = ctx.enter_context(tc.tile_pool(name="pacc", bufs=2))   # per-tile accum
            pout = ctx.enter_context(tc.tile_pool(name="pout", bufs=3))   # small bounces
            pmm = ctx.enter_context(tc.tile_pool(name="pmm", bufs=6))     # dense lhsT tiles
            ppsA = ctx.enter_context(tc.tile_pool(name="ppsA", bufs=2, space="PSUM"))
            ppsT = ctx.enter_context(tc.tile_pool(name="ppsT", bufs=3, space="PSUM"))

            ident = const.tile([P, P], bf16)
            make_identity(nc, ident[:])
            zt = const.tile([P, HD], bf16)
            nc.gpsimd.memset(zt[:], 0)
            padrow = const.tile([1, W1C], bf16)
            nc.gpsimd.memset(padrow[:], 0)
            nc.vector.memset(padrow[:, HD:HD + H], BIG)
            padrow2 = const.tile([1, W2C], bf16)
            nc.gpsimd.memset(padrow2[:], 0)
            nc.vector.memset(padrow2[:, HD:HD + 1], BIG)
            b1_sb = const.tile([P, HD], f32)
            nc.scalar.dma_start(b1_sb[:], b1r[:, :])
            b2_sb = const.tile([P, HD], f32)
            nc.scalar.dma_start(b2_sb[:], b2r[:, :])
            w1_sb = const.tile([P, W1C], bf16)
            nc.scalar.dma_start(w1_sb[:], wc1[:, :])
            w2_sb = []
            for k in range(3):
                w_ = const.tile([P, W2C], bf16, name=f"w2_{k}")
                nc.scalar.dma_start(w_[:], wc2[k * P:(k + 1) * P, :])
                w2_sb.append(w_)
            xT_sb = const.tile([P, RPC], bf16)
            nc.sync.dma_start(xT_sb[:], xT[:, :])
            s1_sb = io.tile([P, S], i32)
            nc.sync.dma_start(s1_sb[:], slots1[:, :])
            s2_sb = io.tile([P, S], i32)
            nc.sync.dma_start(s2_sb[:], slots2[:, :])
            e1i_sb = io.tile([P, NT], i32)
            nc.sync.dma_start(e1i_sb[:], er1i[:, :])
            gsi_sb = io.tile([P, NT], i32)
            nc.sync.dma_start(gsi_sb[:], g2si[:, :])
            er_sb = io.tile([P, NT * H], bf16)
            er2_sb = io.tile([P, NT], bf16)
            gp_sb = io.tile([P, 3 * NB], f32)

            # ---------------- phase A: dense 1 + zero g2p ----------------
            for i in range(NT):
                ps = ppsA.tile([P, W1C], f32)
                nc.tensor.matmul(ps[:], xT_sb[:, i * P:(i + 1) * P], w1_sb[:],
                                 start=True, stop=True)
                ob = pout.tile([P, W1C], bf16)
                nc.vector.tensor_copy(ob[:], ps[:])
                nc.sync.dma_start(t1loc[i * P:(i + 1) * P, :], ob[:])
            for i in range(NB + 1):
                nc.scalar.dma_start(g2p[i * P:(i + 1) * P, :], zt[:])

            tc.strict_bb_all_engine_barrier()
            nc.gpsimd.collective_compute(
                "AllGather", mybir.AluOpType.bypass, replica_groups=groups,
                ins=[t1loc[:, :].opt()], outs=[T1[:, :].opt()])
            tc.strict_bb_all_engine_barrier()
            nc.sync.dma_start(T1[cfg.PAD1:cfg.PAD1 + 1, :], padrow[:])
            if debug:
                nc.scalar.dma_start(dbgT1[:, :], T1[:, :])
            tc.strict_bb_all_engine_barrier()

            # ---------------- phase B: layer-1 aggregation ----------------
            for t in range(NT):
                nc.gpsimd.indirect_dma_start(
                    out=ap3(er_sb, t * H, [(1, H)]),
                    out_offset=None, in_=T1[:, :],
                    in_offset=bass.IndirectOffsetOnAxis(
                        ap=e1i_sb[:, t:t + 1], axis=0),
                    element_offset=HD + H)

            def agg_layer(layer):
                W = W1C if layer == 1 else W2C
                nH = H if layer == 1 else 1
                tbl = T1 if layer == 1 else T2
                s_sb = s1_sb if layer == 1 else s2_sb
                bias = b1_sb if layer == 1 else b2_sb
                for (t, o0, K, first, last) in steps:
                    G = pg.tile([P, K * W], bf16)
                    for j in range(K):
                        nc.gpsimd.indirect_dma_start(
                            out=ap3(G, j * W, [(1, W)]),
                            out_offset=None, in_=tbl[:, :],
                            in_offset=bass.IndirectOffsetOnAxis(
                                ap=s_sb[:, o0 + j:o0 + j + 1], axis=0))
                    el = ap3(G, HD, [(W, K), (1, nH)])
                    if layer == 1:
                        erb = ap3(er_sb, t * H, [(0, K), (1, H)])
                    else:
                        erb = ap3(er2_sb, t, [(0, K), (1, 1)])
                    E = psc.tile([P, K * nH], f32)
                    nc.vector.tensor_tensor(out=E[:], in0=el, in1=erb, op=ADD)
                    E2 = psc.tile([P, K * nH], f32)
                    nc.vector.scalar_tensor_tensor(out=E2[:], in0=E[:], scalar=0.2,
                                                   in1=E[:], op0=MUL, op1=MAX)
                    A = psc.tile([P, K * nH], f32)
                    nc.scalar.activation(A[:], E2[:], EXP)
                    sK = psc.tile([P, nH], f32)
                    nc.vector.tensor_reduce(out=sK[:],
                                            in_=ap3(A, 0, [(1, nH), (nH, K)]),
                                            axis=AX, op=ADD)
                    Ab = psc.tile([P, K * nH], bf16)
                    nc.vector.tensor_copy(Ab[:], A[:])
                    PR = pw.tile([P, K * HD], bf16)
                    if layer == 1:
                        a_bc = ap3(Ab, 0, [(H, K), (1, H), (0, P)])
                    else:
                        a_bc = ap3(Ab, 0, [(1, K), (0, 3), (0, P)])
                    nc.vector.tensor_tensor(
                        out=PR[:],
                        in0=ap3(G, 0, [(W, K), (P, 3), (1, P)]),
                        in1=a_bc,
                        op=MUL)
                    if first:
                        r_acc = pacc.tile([P, HD], f32)
                        s_acc = pacc.tile([P, nH], f32)
                        agg_layer.cur = (r_acc, s_acc)
                    r_acc, s_acc = agg_layer.cur
                    if first:
                        nc.vector.tensor_reduce(
                            out=r_acc[:], in_=ap3(PR, 0, [(P, 3), (1, P), (HD, K)]),
                            axis=AX, op=ADD)
                        nc.vector.tensor_copy(s_acc[:], sK[:])
                    else:
                        rt = psc.tile([P, HD], f32)
                        nc.vector.tensor_reduce(
                            out=rt[:], in_=ap3(PR, 0, [(P, 3), (1, P), (HD, K)]),
                            axis=AX, op=ADD)
                        nc.vector.tensor_tensor(out=r_acc[:], in0=r_acc[:],
                                                in1=rt[:], op=ADD)
                        nc.vector.tensor_tensor(out=s_acc[:], in0=s_acc[:],
                                                in1=sK[:], op=ADD)
                    if not last:
                        continue
                    rc = psc.tile([P, nH], f32)
                    nc.vector.tensor_scalar_max(out=rc[:], in0=s_acc[:],
                                                scalar1=1e-20)
                    nc.vector.reciprocal(rc[:], rc[:])
                    q2 = psc.tile([P, HD], f32)
                    if layer == 1:
                        q = psc.tile([P, HD], f32)
                        nc.vector.tensor_tensor(out=q[:], in0=r_acc[:],
                                                in1=ap3(rc, 0, [(1, H), (0, P)]),
                                                op=MUL)
                        nc.vector.tensor_tensor(out=q2[:], in0=q[:], in1=bias[:],
                                                op=ADD)
                    else:
                        nc.vector.scalar_tensor_tensor(out=q2[:], in0=r_acc[:],
                                                       scalar=rc[:, 0:1],
                                                       in1=bias[:], op0=MUL, op1=ADD)
                    gtile = psc.tile([P, HD], bf16)
                    nc.scalar.activation(gtile[:], q2[:], RELU)
                    if layer == 1:
                        for k in range(3):
                            psT = ppsT.tile([P, P], bf16)
                            nc.tensor.transpose(psT[:], gtile[:, k * P:(k + 1) * P],
                                                ident[:])
                            tb = pout.tile([P, P], bf16)
                            nc.vector.tensor_copy(tb[:], psT[:])
                            nc.sync.dma_start(
                                dap(gT_d, (k * P) * RPC + t * P, [(RPC, P), (1, P)]),
                                tb[:])
                    else:
                        nc.gpsimd.indirect_dma_start(
                            out=g2p[:, :],
                            out_offset=bass.IndirectOffsetOnAxis(
                                ap=gsi_sb[:, t:t + 1], axis=0),
                            in_=gtile[:], in_offset=None)

            agg_layer(1)

            tc.strict_bb_all_engine_barrier()

            # ---------------- phase C: dense 2 ----------------
            for i in range(NT):
                ps = ppsA.tile([P, W2C], f32)
                for k in range(3):
                    lt = pmm.tile([P, P], bf16)
                    nc.sync.dma_start(lt[:], dap(gT_d, (k * P) * RPC + i * P,
                                                 [(RPC, P), (1, P)]))
                    nc.tensor.matmul(ps[:], lt[:], w2_sb[k][:],
                                     start=(k == 0), stop=(k == 2))
                ob = pout.tile([P, W2C], bf16)
                nc.vector.tensor_copy(ob[:], ps[:])
                nc.sync.dma_start(t2loc[i * P:(i + 1) * P, :], ob[:])

            tc.strict_bb_all_engine_barrier()
            nc.gpsimd.collective_compute(
                "AllGather", mybir.AluOpType.bypass, replica_groups=groups,
                ins=[t2loc[:, :].opt()], outs=[T2[0:NPAD, :].opt()])
            tc.strict_bb_all_engine_barrier()
            nc.sync.dma_start(T2[NPAD:NPAD + 1, :], padrow2[:])
            nc.scalar.dma_start(
                er2_sb[:], dap(t2loc, HD + 1, [(W2C, P), (P * W2C, NT)]))
            if debug:
                nc.scalar.dma_start(dbgT2[:, :], T2[:, :])
            tc.strict_bb_all_engine_barrier()

            # ---------------- phase D: layer-2 aggregation ----------------
            agg_layer(2)

            tc.strict_bb_all_engine_barrier()

            # ---------------- phase E: pooling ----------------
            for blk in range(NB):
                bt = pout.tile([P, HD], bf16)
                nc.sync.dma_start(bt[:], dap(g2p, blk * P * HD, [(HD, P), (1, HD)]))
                for k in range(3):
                    psT = ppsT.tile([P, P], bf16)
                    nc.tensor.transpose(psT[:], bt[:, k * P:(k + 1) * P], ident[:])
                    nc.vector.tensor_reduce(
                        out=gp_sb[:, blk * 3 + k:blk * 3 + k + 1],
                        in_=psT[:], axis=AX, op=MAX)
            nc.sync.dma_start(gp_out[:, :], gp_sb[:])
            if debug:
                nc.scalar.dma_start(dbgG[:, :], gT_d[:, :])
                nc.scalar.dma_start(dbgP[:, :], g2p[:, :])

    nc.compile()
    return nc


_CACHE = {}


def _get_program(cfg, sched, debug=False):
    key = (cfg.N, cfg.E, cfg.B, tuple(sched["Kt"].tolist()), debug)
    if key not in _CACHE:
        _CACHE[key] = _build_program(cfg, sched, debug=debug)
    return _CACHE[key]


def run_gat_device(cfg, sched, node_feat, W1, al1, ar1, b1, W2, al2, ar2, b2,
                   debug=False):
    """Run the fused device program; returns gpool [B, 384] float32."""
    import ml_dtypes
    from concourse.bass_utils import run_bass_kernel_spmd
    bf = ml_dtypes.bfloat16
    f32 = np.float32
    N, NC, NLOC, RPC, D, H = cfg.N, cfg.NC, cfg.NLOC, cfg.RPC, cfg.D, cfg.H

    Wl1 = np.stack([W1[:, h * D:(h + 1) * D] @ al1[h] for h in range(H)], axis=1)
    Wr1 = np.stack([W1[:, h * D:(h + 1) * D] @ ar1[h] for h in range(H)], axis=1)
    wc1 = np.concatenate([W1, Wl1, Wr1], axis=1).astype(bf)          # [128, 390]
    Wl2 = (W2 @ al2[0])[:, None]
    Wr2 = (W2 @ ar2[0])[:, None]
    wc2 = np.concatenate([W2, Wl2, Wr2], axis=1).astype(bf)          # [384, 386]
    b1rep = np.broadcast_to(b1.astype(f32), (P, cfg.HD)).copy()
    b2rep = np.broadcast_to(b2.astype(f32), (P, cfg.HD)).copy()

    xpad = np.zeros((cfg.NPAD, D), np.float32)
    xpad[:N] = node_feat
    xT = np.ascontiguousarray(xpad.T.astype(bf))                     # [128, NPAD]

    in_maps = []
    for c in range(NC):
        in_maps.append({
            "xT": np.ascontiguousarray(xT[:, c * RPC:(c + 1) * RPC]),
            "wc1": wc1, "wc2": wc2, "b1r": b1rep, "b2r": b2rep,
            "slots1": sched["slots1"][c], "slots2": sched["slots2"][c],
            "er1i": sched["er1idx"][c], "g2si": sched["g2sidx"][c],
        })
    nc = _get_program(cfg, sched, debug=debug)
    res = run_bass_kernel_spmd(nc, in_maps, list(range(NC)))
    if debug:
        run_gat_device.dbg = res.results

    gpool = np.zeros((cfg.B, cfg.HD), np.float32)
    bg = sched["block_graph"]
    for c in range(NC):
        gp = res.results[c]["gp"]                                    # [128, 3*NB]
        for blk in range(cfg.NB):
            g = int(bg[c, blk])
            if g < 0:
                continue
            arr = gp[:, blk * 3:blk * 3 + 3].T.reshape(cfg.HD)       # feat-major
            np.maximum(gpool[g], arr, out=gpool[g])
    return gpool


# ---------------------------------------------------------------- full model
def _maxpool(x, k, s):
    T = x.shape[2]
    nt = (T - k) // s + 1
    out = x[:, :, :nt * s:s].copy()
    for j in range(1, k):
        np.maximum(out, x[:, :, j:j + nt * s:s], out=out)
    return out


def _conv1d(x, w, b):
    T = x.shape[2]
    out = np.matmul(w[:, :, 0], x[:, :, 0:T - 2])
    out += np.matmul(w[:, :, 1], x[:, :, 1:T - 1])
    out += np.matmul(w[:, :, 2], x[:, :, 2:T])
    return out + b[None, :, None]


LAST_EXEC_NS = 0


def kernel(node_feat, src, dst, graph_ids, pad_dmap,
           W1, al1, ar1, b1, W2, al2, ar2, b2,
           fc_g1_w, fc_g1_b, conv1_w, conv1_b, conv2_w, conv2_b,
           conv3_w, conv3_b, tf_w, tf_b, w1,
           fc1_w, fc1_b, fc2_w, fc2_b, out_w, out_b):
    import time
    global LAST_EXEC_NS
    f32 = np.float32
    node_feat = np.asarray(node_feat, f32)
    src = np.asarray(src, np.int64)
    dst = np.asarray(dst, np.int64)
    graph_ids = np.asarray(graph_ids, np.int64)
    pad_dmap = np.asarray(pad_dmap, f32)
    W1, al1, ar1, b1 = (np.asarray(a, f32) for a in (W1, al1, ar1, b1))
    W2, al2, ar2, b2 = (np.asarray(a, f32) for a in (W2, al2, ar2, b2))

    cfg = Cfg(node_feat.shape[0], src.shape[0], 32)
    sched = _schedule(cfg, src, dst, graph_ids)
    t0 = time.time()
    gpool = run_gat_device(cfg, sched, node_feat, W1, al1, ar1, b1,
                           W2, al2, ar2, b2)
    LAST_EXEC_NS = int((time.time() - t0) * 1e9)

    g1 = np.maximum(gpool @ np.asarray(fc_g1_w, f32) + np.asarray(fc_g1_b, f32), 0.0)

    x = pad_dmap[:, 0].transpose(0, 2, 1)
    f = _maxpool(_conv1d(x, np.asarray(conv1_w, f32), np.asarray(conv1_b, f32)), 3, 3)
    f = _maxpool(_conv1d(f, np.asarray(conv2_w, f32), np.asarray(conv2_b, f32)), 3, 3)
    f = _conv1d(f, np.asarray(conv3_w, f32), np.asarray(conv3_b, f32))
    f = f.max(axis=2)
    seq1 = np.maximum(f @ np.asarray(tf_w, f32) + np.asarray(tf_b, f32), 0.0)

    wv = 1.0 / (1.0 + np.exp(-np.asarray(w1, f32)[0]))
    gc = (1.0 - wv) * g1 + wv * seq1
    gc = np.maximum(gc @ np.asarray(fc1_w, f32) + np.asarray(fc1_b, f32), 0.0)
    gc = np.maximum(gc @ np.asarray(fc2_w, f32) + np.asarray(fc2_b, f32), 0.0)
    o = np.maximum(gc @ np.asarray(out_w, f32) + np.asarray(out_b, f32), 0.0)
    o = o - o.max(axis=1, keepdims=True)
    eo = np.exp(o)
    return (eo / eo.sum(axis=1, keepdims=True)).astype(f32)
